# revision 15
# baseline (speedup 1.0000x reference)
"""Trainium2 Bass kernel for the BoW language model head problem.

Model (per reference):
    emb = wte[x] + wpe            (B,T,C)
    h   = emb + cumsum(emb)/[1..T]
    h   = h + tanh(h@w_fc+b_fc)@w_proj + b_proj
    out = h @ w_head + b_head     (B,T,V)

Shapes: B=4, T=2048, V=50257, C=512, H=2048.

Sharding (8 cores): core i computes batch i//2 and vocab half i%2.
The pre-head compute (embedding, causal BoW, MLP) is split between the
two cores of a vocab pair: each handles half the tokens (its own half
first in its local block order), then an AllReduce over the pair plus
a subtract reconstructs the peer's half.  Head groups are processed
own-half-first so the collective overlaps ~2 groups of head matmuls.
All matmul operands are bf16 (full PE rate); PSUM accumulates fp32.
Output logits are written bf16 and widened to fp32 on the host, which
halves the dominant DMA stream (the 1.6 GB logits tensor).
"""

import numpy as np

import concourse.bacc as bacc
import concourse.bass as bass
import concourse.mybir as mybir
import concourse.tile as tile
from concourse.bass_utils import run_bass_kernel_spmd

P = 128
B, T, V, C, H = 4, 2048, 50257, 512, 2048
NBLK = T // P          # 16 token blocks
NLOC = NBLK // 2       # 8 local token blocks per core (pair-split pre-head)
TLOC = NLOC * P        # 1024 local tokens
NCC = C // P           # 4 C chunks
NHC = H // P           # 16 H chunks
TG = 512               # token group width (MLP moving dim)
VT = 512               # vocab tile width
NVT = 50               # vocab tiles per half
GSZ = 5                # vocab tiles per head group
NG = NVT // GSZ        # 10 head groups
VHALF_PAD = NVT * VT   # 25600
VSPLIT = (V + 1) // 2  # 25129: half0 = [:VSPLIT], half1 = [VSPLIT:]
VLAST = VSPLIT - (NVT - 1) * VT  # 41 live cols in the final vocab tile

F32 = mybir.dt.float32
F32R = mybir.dt.float32r
BF16 = mybir.dt.bfloat16
I32 = mybir.dt.int32


def _build_nc():
    nc = bacc.Bacc(None, target_bir_lowering=False, debug=True,
                   num_swdge_queues=4, num_devices=8)

    x_idx = nc.dram_tensor("x_idx", [P, NBLK], I32, kind="ExternalInput")
    wte = nc.dram_tensor("wte", [V, C], BF16, kind="ExternalInput")
    wpe_e = nc.dram_tensor("wpe_e", [P, NBLK, C], BF16, kind="ExternalInput")
    w_fc = nc.dram_tensor("w_fc", [P, NCC, H], BF16, kind="ExternalInput")
    w_proj = nc.dram_tensor("w_proj", [P, NHC, C], BF16, kind="ExternalInput")
    w_head = nc.dram_tensor("w_head", [P, NVT, NCC, VT], BF16,
                            kind="ExternalInput")
    b_fc2d = nc.dram_tensor("b_fc2d", [P, NHC], F32, kind="ExternalInput")
    b_proj2d = nc.dram_tensor("b_proj2d", [P, NCC], F32, kind="ExternalInput")
    recip = nc.dram_tensor("recip", [P, NLOC], F32, kind="ExternalInput")
    mask = nc.dram_tensor("mask", [P, 1], F32, kind="ExternalInput")
    tri = nc.dram_tensor("tri", [P, P], BF16, kind="ExternalInput")
    ones = nc.dram_tensor("ones", [P, P], BF16, kind="ExternalInput")
    oneth = nc.dram_tensor("oneth", [P, P], BF16, kind="ExternalInput")
    ident = nc.dram_tensor("ident", [P, P], F32R, kind="ExternalInput")
    out = nc.dram_tensor("out", [T, VHALF_PAD], BF16, kind="ExternalOutput")

    with tile.TileContext(nc) as tc:
        with tc.tile_pool(name="consts", bufs=1) as consts, \
             tc.tile_pool(name="hfp", bufs=1) as hfp, \
             tc.tile_pool(name="peerp", bufs=1) as peerp, \
             tc.tile_pool(name="htp", bufs=1) as htp, \
             tc.tile_pool(name="wmats", bufs=1) as wmats, \
             tc.tile_pool(name="whp", bufs=3) as whp, \
             tc.tile_pool(name="ccdr", bufs=1, space="DRAM") as ccdr:
            idx_sb = consts.tile([P, NBLK], I32, tag="idx")
            nc.sync.dma_start(out=idx_sb[:], in_=x_idx[:])
            recip_sb = consts.tile([P, NLOC], F32, tag="recip")
            nc.sync.dma_start(out=recip_sb[:], in_=recip[:])
            mask_sb = consts.tile([P, 1], F32, tag="mask")
            nc.sync.dma_start(out=mask_sb[:], in_=mask[:])
            bfc_sb = consts.tile([P, NHC], F32, tag="bfc")
            nc.sync.dma_start(out=bfc_sb[:], in_=b_fc2d[:])
            bproj_sb = consts.tile([P, NCC], F32, tag="bproj")
            nc.sync.dma_start(out=bproj_sb[:], in_=b_proj2d[:])
            tri_sb = consts.tile([P, P], BF16, tag="tri")
            nc.sync.dma_start(out=tri_sb[:], in_=tri[:])
            ones_sb = consts.tile([P, P], BF16, tag="ones")
            nc.sync.dma_start(out=ones_sb[:], in_=ones[:])
            oneth_sb = consts.tile([P, P], BF16, tag="oneth")
            nc.sync.dma_start(out=oneth_sb[:], in_=oneth[:])
            ident_sb = consts.tile([P, P], F32R, tag="ident")
            nc.sync.dma_start(out=ident_sb[:], in_=ident[:])

            # hT holds this core's half pre-MLP (C-major); hFloc post-MLP.
            hT = htp.tile([P, NCC, TLOC], BF16, tag="hT")
            hFloc = hfp.tile([P, NCC, TLOC], BF16, tag="hFloc")
            peer_hF = peerp.tile([P, NCC, TLOC], BF16, tag="peer")
            cc_in = ccdr.tile([P, NCC, TLOC], BF16, tag="cc_in")
            cc_red = ccdr.tile([P, NCC, TLOC], BF16, tag="cc_red")

            def load_group(g):
                whg = whp.tile([P, GSZ, NCC, VT], BF16, tag="whg")
                nc.sync.dma_start(out=whg[:],
                                  in_=w_head[:, g * GSZ:(g + 1) * GSZ, :, :])
                return whg

            # ---------------- Phase B: embedding + causal BoW ----------------
            # The per-block prefix sums live entirely in PSUM: the ones/tri
            # matmuls reduce over partitions anyway, so each block's bank
            # accumulates mask*sum(other half) (via s0) + sum of preceding own
            # blocks + tri@E_j, with no serial vector chain.
            with tc.tile_pool(name="embp", bufs=1) as embp, \
                 tc.tile_pool(name="gp", bufs=NBLK) as gp, \
                 tc.tile_pool(name="s0p", bufs=1) as s0p, \
                 tc.tile_pool(name="hap", bufs=3) as hap, \
                 tc.tile_pool(name="psb", bufs=6, space="PSUM") as psb, \
                 tc.tile_pool(name="pst", bufs=2, space="PSUM") as pst:
                # wpe in E layout, one DMA; first on the SP queue so the
                # embedding adds are never starved by weight prefetches.
                wpe_sb = embp.tile([P, NBLK, C], BF16, tag="wpe")
                nc.sync.dma_start(out=wpe_sb[:], in_=wpe_e[:])

                # MLP weights + first head groups prefetch: independent of
                # phase B; they stream in behind wpe while it runs.
                wfc_sb = wmats.tile([P, NCC, H], BF16, tag="wfc")
                nc.sync.dma_start(out=wfc_sb[:], in_=w_fc[:])
                wproj_sb = wmats.tile([P, NHC, C], BF16, tag="wproj")
                nc.sync.dma_start(out=wproj_sb[:], in_=w_proj[:])
                whg_tiles = {g: load_group(g) for g in range(3)}

                E = embp.tile([P, NBLK, C], BF16, tag="E")
                # all gathers issued back-to-back (descriptor gen on GpSimd
                # is the pacing step); OTHER half (slots 8..15) first: their
                # masked sum s0 seeds every local block's prefix.
                gorder = list(range(NLOC, NBLK)) + list(range(NLOC))
                g_tiles = {}
                for j in gorder:
                    g = gp.tile([P, C], BF16, tag="g")
                    nc.gpsimd.indirect_dma_start(
                        out=g[:], out_offset=None, in_=wte[:],
                        in_offset=bass.IndirectOffsetOnAxis(
                            ap=idx_sb[:, j:j + 1], axis=0),
                    )
                    g_tiles[j] = g
                for j in gorder:
                    nc.vector.tensor_add(E[:, j, :], g_tiles[j][:],
                                         wpe_sb[:, j, :])

                # s0 = mask * sum(other half)  (every partition holds the
                # total; the masked scale happens on the PSUM read)
                pb_o = psb.tile([P, C], F32, tag="bank")
                for j in range(NLOC, NBLK):
                    nc.tensor.matmul(pb_o[:], lhsT=ones_sb[:], rhs=E[:, j, :],
                                     start=(j == NLOC), stop=(j == NBLK - 1))
                s0 = s0p.tile([P, C], BF16, tag="s0")
                nc.scalar.activation(s0[:], pb_o[:],
                                     mybir.ActivationFunctionType.Copy,
                                     scale=mask_sb[:, :1])

                for j in range(NLOC):
                    pb = psb.tile([P, C], F32, tag="bank")
                    # oneth = ones/128 undoes the partition-broadcast of s0
                    nc.tensor.matmul(pb[:], lhsT=oneth_sb[:], rhs=s0[:],
                                     start=True, stop=False)
                    for k in range(j):
                        nc.tensor.matmul(pb[:], lhsT=ones_sb[:],
                                         rhs=E[:, k, :],
                                         start=False, stop=False)
                    nc.tensor.matmul(pb[:], lhsT=tri_sb[:], rhs=E[:, j, :],
                                     start=False, stop=True)
                    tmpb = hap.tile([P, C], F32, tag="tmpb")
                    nc.scalar.activation(tmpb[:], pb[:],
                                         mybir.ActivationFunctionType.Copy,
                                         scale=recip_sb[:, j:j + 1])
                    hA = hap.tile([P, C], F32R, tag="hA")
                    nc.vector.tensor_add(hA[:], tmpb[:], E[:, j, :])
                    for c in range(NCC):
                        pt = pst.tile([P, P], F32R, tag="tr")
                        nc.tensor.transpose(pt[:], hA[:, c * P:(c + 1) * P],
                                            ident_sb[:])
                        if c % 2:
                            nc.scalar.activation(
                                hT[:, c, j * P:(j + 1) * P], pt[:],
                                mybir.ActivationFunctionType.Copy)
                        else:
                            nc.vector.tensor_copy(
                                hT[:, c, j * P:(j + 1) * P], pt[:])

            # ---------------- Phase C: MLP (local half) ----------------
            with tc.tile_pool(name="ap_", bufs=NHC) as ap_, \
                 tc.tile_pool(name="ctmp", bufs=3) as ctmp, \
                 tc.tile_pool(name="psfc", bufs=2, space="PSUM") as psfc, \
                 tc.tile_pool(name="pspj", bufs=1, space="PSUM") as pspj:
                for gidx in range(TLOC // TG):
                    gsl = slice(gidx * TG, (gidx + 1) * TG)
                    a_tiles = []
                    for hc in range(NHC):
                        pfc = psfc.tile([P, TG], F32, tag="fc")
                        for c in range(NCC):
                            nc.tensor.matmul(
                                pfc[:], lhsT=wfc_sb[:, c, hc * P:(hc + 1) * P],
                                rhs=hT[:, c, gsl],
                                start=(c == 0), stop=(c == NCC - 1))
                        a = ap_.tile([P, TG], BF16, tag="a")
                        nc.scalar.activation(a[:], pfc[:],
                                             mybir.ActivationFunctionType.Tanh,
                                             bias=bfc_sb[:, hc:hc + 1])
                        a_tiles.append(a)
                    pproj = pspj.tile([P, NCC, TG], F32, tag="proj")
                    for cc in range(NCC):
                        for hc in range(NHC):
                            nc.tensor.matmul(
                                pproj[:, cc, :],
                                lhsT=wproj_sb[:, hc, cc * P:(cc + 1) * P],
                                rhs=a_tiles[hc][:],
                                start=(hc == 0), stop=(hc == NHC - 1))
                        tmpc = ctmp.tile([P, TG], F32, tag="tmpc")
                        nc.scalar.activation(tmpc[:], pproj[:, cc, :],
                                             mybir.ActivationFunctionType.Identity,
                                             bias=bproj_sb[:, cc:cc + 1])
                        nc.vector.tensor_add(hFloc[:, cc, gsl], tmpc[:],
                                             hT[:, cc, gsl])

            # ---------------- pair AllReduce of hF ----------------
            nc.sync.dma_start(out=cc_in[:], in_=hFloc[:])
            nc.gpsimd.collective_compute(
                "AllReduce",
                mybir.AluOpType.add,
                replica_groups=[[0, 1], [2, 3], [4, 5], [6, 7]],
                ins=[cc_in[:].opt()],
                outs=[cc_red[:].opt()],
            )

            # ---------------- Phase D: head ----------------
            with tc.tile_pool(name="sump", bufs=1) as sump, \
                 tc.tile_pool(name="stp", bufs=3) as stp, \
                 tc.tile_pool(name="pso", bufs=8, space="PSUM") as pso:
                # peer half = allreduce sum - own half (per-cc so the head's
                # first peer matmuls, which read cc=0, unblock earliest)
                sm = sump.tile([P, NCC, TLOC], BF16, tag="sum")
                for cc in range(NCC):
                    nc.sync.dma_start(out=sm[:, cc, :], in_=cc_red[:, cc, :])
                    nc.vector.tensor_tensor(
                        out=peer_hF[:, cc, :], in0=sm[:, cc, :],
                        in1=hFloc[:, cc, :], op=mybir.AluOpType.subtract)

                def compute_chunk(g, own):
                    whg = whg_tiles[g]
                    src_h = hFloc if own else peer_hF
                    # live columns: the final vocab tile is mostly padding
                    widths = [VT] * GSZ
                    if g == NG - 1:
                        widths[GSZ - 1] = VLAST
                    cols = (GSZ - 1) * VT + widths[GSZ - 1]
                    for jloc in range(NLOC):
                        j = jloc if own else NLOC + jloc
                        jj = jloc * P
                        jsl = slice(j * P, (j + 1) * P)
                        psums = []
                        for _vt in range(GSZ):
                            po = pso.tile([P, VT], F32, tag="po")
                            psums.append(po)
                        for c in range(NCC):
                            for vt in range(GSZ):
                                w = widths[vt]
                                nc.tensor.matmul(
                                    psums[vt][:, :w],
                                    lhsT=src_h[:, c, jj:jj + P],
                                    rhs=whg[:, vt, c, :w],
                                    start=(c == 0), stop=(c == NCC - 1))
                        st = stp.tile([P, GSZ * VT], BF16, tag="stage")
                        for vt in range(GSZ):
                            w = widths[vt]
                            dst = st[:, vt * VT:vt * VT + w]
                            if (vt + j) % 2:
                                nc.scalar.activation(
                                    dst, psums[vt][:, :w],
                                    mybir.ActivationFunctionType.Copy)
                            else:
                                nc.vector.tensor_copy(dst, psums[vt][:, :w])
                        nc.sync.dma_start(
                            out=out[jsl,
                                    g * GSZ * VT:g * GSZ * VT + cols],
                            in_=st[:, :cols])

                # Three own-half chunks run while the AllReduce is in flight;
                # each later group is prefetched three chunks ahead.
                order = [(0, True), (1, True), (2, True),
                         (0, False), (1, False), (2, False)]
                for g in range(3, NG):
                    order += [(g, True), (g, False)]
                next_load = 3
                for g, own in order:
                    compute_chunk(g, own)
                    if not own:
                        del whg_tiles[g]
                        if next_load < NG:
                            whg_tiles[next_load] = load_group(next_load)
                            next_load += 1
    nc.compile()
    return nc


_NC = None


def _get_nc():
    global _NC
    if _NC is None:
        _NC = _build_nc()
    return _NC


def _bf16():
    import ml_dtypes
    return ml_dtypes.bfloat16


def make_in_maps(x, wte, wpe, w_fc, b_fc, w_proj, b_proj, w_head, b_head):
    bf16 = _bf16()
    x = np.asarray(x).astype(np.int32)
    wte_b = np.asarray(wte, dtype=np.float32).astype(bf16)
    wpe_b = np.asarray(wpe, dtype=np.float32).astype(bf16)
    wfc_b = np.asarray(w_fc, dtype=np.float32).astype(bf16)
    wproj_b = np.asarray(w_proj, dtype=np.float32).astype(bf16)
    whead_b = np.asarray(w_head, dtype=np.float32).astype(bf16)
    b_fc = np.asarray(b_fc, dtype=np.float32)
    b_proj = np.asarray(b_proj, dtype=np.float32)

    # [C, VHALF_PAD] padded halves -> [P, NVT, NCC, VT] tile layout
    wh_halves = []
    for vh in range(2):
        lo = vh * VSPLIT
        hi = min(V, lo + VSPLIT)
        pad = np.zeros((C, VHALF_PAD), bf16)
        pad[:, :hi - lo] = whead_b[:, lo:hi]
        tilew = pad.reshape(NCC, P, NVT, VT).transpose(1, 2, 0, 3)
        wh_halves.append(np.ascontiguousarray(tilew))

    # per-half block permutation: own half's blocks first
    orders = [list(range(vh * NLOC, vh * NLOC + NLOC)) +
              list(range((1 - vh) * NLOC, (1 - vh) * NLOC + NLOC))
              for vh in range(2)]
    wpe_blocks = wpe_b.reshape(NBLK, P, C)
    wpe_perms = [np.ascontiguousarray(wpe_blocks[o].transpose(1, 0, 2))
                 for o in orders]  # [P, NBLK, C]
    t_idx = np.arange(1, T + 1, dtype=np.float32)
    recip_full = (1.0 / t_idx).reshape(NBLK, P).T  # [P, NBLK] global
    recips = [np.ascontiguousarray(recip_full[:, vh * NLOC:(vh + 1) * NLOC])
              for vh in range(2)]
    b_fc2d = np.ascontiguousarray(b_fc.reshape(NHC, P).T)
    b_proj2d = np.ascontiguousarray(b_proj.reshape(NCC, P).T)
    tri = np.triu(np.ones((P, P), np.float32)).astype(bf16)
    ones = np.ones((P, P), bf16)
    oneth = np.full((P, P), 1.0 / P, bf16)
    ident = np.eye(P, dtype=np.float32)
    wfc_t = np.ascontiguousarray(wfc_b.reshape(NCC, P, H).transpose(1, 0, 2))
    wproj_t = np.ascontiguousarray(
        wproj_b.reshape(NHC, P, C).transpose(1, 0, 2))

    in_maps = []
    for core in range(8):
        b = core // 2
        vh = core % 2
        x_idx = np.ascontiguousarray(x[b].reshape(NBLK, P)[orders[vh]].T)
        in_maps.append({
            "x_idx": x_idx,
            "wte": wte_b,
            "wpe_e": wpe_perms[vh],
            "w_fc": wfc_t,
            "w_proj": wproj_t,
            "w_head": wh_halves[vh],
            "b_fc2d": b_fc2d,
            "b_proj2d": b_proj2d,
            "recip": recips[vh],
            "mask": np.full((P, 1), float(vh), np.float32),
            "tri": tri,
            "ones": ones,
            "oneth": oneth,
            "ident": ident,
        })
    return in_maps


def kernel(x, wte, wpe, w_fc, b_fc, w_proj, b_proj, w_head, b_head):
    b_head = np.asarray(b_head, dtype=np.float32)
    in_maps = make_in_maps(x, wte, wpe, w_fc, b_fc, w_proj, b_proj,
                           w_head, b_head)
    nc = _get_nc()
    res = run_bass_kernel_spmd(nc, in_maps, core_ids=list(range(8)))

    logits = np.empty((B, T, V), np.float32)
    for core in range(8):
        b = core // 2
        vh = core % 2
        lo = vh * VSPLIT
        hi = min(V, lo + VSPLIT)
        co = np.asarray(res.results[core]["out"][:, :hi - lo],
                        dtype=np.float32)
        # rows are in local block order: own half first
        logits[b, vh * TLOC:vh * TLOC + TLOC, lo:hi] = co[:TLOC]
        logits[b, (1 - vh) * TLOC:(1 - vh) * TLOC + TLOC, lo:hi] = co[TLOC:]
    if b_head.any():
        logits += b_head[None, None, :]
    return logits
